# revision 1
# baseline (speedup 1.0000x reference)
"""Trainium2 Bass kernel for nn_BasicTransformerBlock (key-frame cross attention).

Reference computation (B=16 frames, S=1024, C=320, H=8 heads, D=40):
    q = x @ Wq.T ; k = x @ Wk.T ; v = x @ Wv.T
    k, v are taken from frame `kf` only and shared by every frame
    out = softmax(q k^T / sqrt(D)) v     (per frame, per head)
    y = out @ Wo.T + bo

Sharding: data-parallel over frames - 8 cores x 2 frames each. Every core
redundantly computes K/V from the key frame (cheap) so no collectives are
needed; outputs just concatenate.

Per-core design:
  - Everything runs "transposed": xT [C, S] tiles feed projections that
    produce qT/kT [c_out_padded, S] directly; scoresT [t, s] come from
    kT-slices.T @ qT-slices. With this layout softmax needs no reduction
    instruction and no max-subtraction (scores are tiny by construction:
    |score|/sqrt(D) < ~1 for this problem's weight scale).
  - Heads are zero-padded D=40 -> 64 host-side so each head sits at a
    64-aligned partition offset; even/odd heads of a pair can row-pack the
    PE array concurrently via tile_position (0,0)/(64,0).
  - V is stored per-head as [t, {Vpad 64 | ones 64}] blocks so one matmul
    per (head, t-tile) yields psum rows 0-63 = out^T and rows 64-127 = the
    softmax denominator replicated 64x. A cross-quadrant DVE reciprocal
    (psum[64:128] -> sbuf[0:64]) plus one tensor_mul normalizes without any
    partition-broadcast tricks.
  - All matmuls run float32r: full PE rate at N=512 with ~1e-4 relative
    precision. DRAM inputs are declared float32r (same bits as fp32) so
    plain HWDGE DMAs feed them with no cast.
  - ScalarE is the roofline (16.8M exps/core, ~110us min). The emission
    order keeps it fed: a minimal hot head (K/Q head-pair 0 via a small
    "whot" weight tensor + per-t-tile V projection pipelined one step ahead
    of PV), remaining projections spread as extras inside earlier attention
    units, O-projection bias-adds on ScalarE Identity at boundaries where
    it idles anyway. Inputs arrive as one DMA per tensor (c_in padded to
    384 rows so a single 3-level-AP transfer covers all partition chunks),
    minimizing HWDGE ring serialization.
  - PSUM: scores pool 2x[128,1024] (double-buffered, feeds exp) + 4 PV
    accumulators [128,512] per unit = all 8 banks; projection/O-proj psum
    time-shares those pools in allocation order (PE is strictly in-order,
    so slot order must match use order or everything stalls).
  - y^T [C, S] is DMAed out per c_out chunk and un-transposed on the host.
"""

import os
import sys

import numpy as np

try:
    import concourse  # noqa: F401
except ImportError:  # pragma: no cover
    for _p in ("/opt/trn_rl_repo", os.path.dirname(os.path.abspath(__file__))):
        if os.path.isdir(os.path.join(_p, "concourse")):
            sys.path.insert(0, _p)
            break

import concourse.mybir as mybir  # noqa: E402
import concourse.tile as tile  # noqa: E402
from concourse import bacc  # noqa: E402
from concourse import bass_utils  # noqa: E402

F32 = mybir.dt.float32
F32R = mybir.dt.float32r

S = 1024          # sequence length per frame
C = 320           # channels
H = 8             # heads
D = 40            # head dim
DP = 64           # padded head dim
CP = H * DP       # 512, padded channels
NCORES = 8
FPC = 2           # frames per core
SCALE = float(D) ** -0.5

CI = [(0, 128), (128, 128), (256, 64)]    # c_in chunks of 320
CO = [(0, 128), (128, 128), (256, 64)]    # c_out chunks of 320

_NC_CACHE: dict = {}
LAST_RESULTS = None  # set by _run for test harness introspection


def _build(loop_n: int = 1):
    nc = bacc.Bacc("TRN2", target_bir_lowering=False, debug=False)

    CPAD = 384  # c_in padded to 3*128: one 3-level-AP DMA covers all chunks
    xt0 = nc.dram_tensor("xt0", [CPAD, S], F32R, kind="ExternalInput")
    xtf = nc.dram_tensor("xtf", [FPC, CPAD, S], F32R, kind="ExternalInput")
    wkq = nc.dram_tensor("wkq", [CPAD, 2 * CP], F32R, kind="ExternalInput")
    wvp = nc.dram_tensor("wvp", [CPAD, CP], F32R, kind="ExternalInput")
    whot = nc.dram_tensor("whot", [CPAD, 256], F32R, kind="ExternalInput")  # wk_m0|wq_m0
    wo = nc.dram_tensor("wo", [CP, C], F32R, kind="ExternalInput")
    bo = nc.dram_tensor("bo", [CPAD], F32, kind="ExternalInput")
    yt = nc.dram_tensor("yt", [FPC, C, S], F32, kind="ExternalOutput")

    with tile.TileContext(nc) as tc:
        with (
            tc.tile_pool(name="pconst", bufs=1) as pconst,
            tc.tile_pool(name="pqk", bufs=1) as pqk,
            tc.tile_pool(name="pvs", bufs=1) as pvs,
            tc.tile_pool(name="pout", bufs=1) as pout,
            tc.tile_pool(name="ppt", bufs=3) as ppt,
            tc.tile_pool(name="prc", bufs=4) as prc,
            tc.tile_pool(name="py", bufs=2) as py,
            tc.tile_pool(name="psb", bufs=2, space="PSUM") as psb,
            tc.tile_pool(name="psv", bufs=4, space="PSUM") as psv,
        ):
          for it in range(loop_n):
            P = f"{it}_"

            ones_sb = pconst.tile([128, DP], F32, name=f"{P}ones", tag="ones")
            nc.gpsimd.memset(ones_sb[:], 1.0)
            wo_all = pconst.tile([128, 4 * C], F32R, name=f"{P}wo", tag="wo")
            wo_sb = [wo_all[:, cp * C:(cp + 1) * C] for cp in range(4)]

            kTp = [pqk.tile([128, S], F32R, name=f"{P}kTp{m}", tag=f"kTp{m}") for m in range(4)]
            qTp = [
                [pqk.tile([128, S], F32R, name=f"{P}qTp{f}_{m}", tag="qT", bufs=5) for m in range(4)]
                for f in range(FPC)
            ]
            v_sb = [pvs.tile([128, H * 128], F32R, name=f"{P}v{tt}", tag=f"v{tt}") for tt in range(8)]

            def one_dma_tile(pool, nm, dram_ap, width, dt=F32R):
                """[128, 3*width] tile <- [384, width] dram via one 3-level DMA.
                Returns (tile, per-ci views clipped to real partition counts)."""
                t = pool.tile([128, 3 * width], dt, name=f"{P}{nm}", tag=nm)
                nc.sync.dma_start(
                    t[:].rearrange("p (c w) -> p c w", w=width),
                    dram_ap.rearrange("(c p) w -> p c w", p=128),
                )
                return t, [t[0:cn, ci * width:ci * width + width] for ci, (cs, cn) in enumerate(CI)]

            def proj_full(dst, w_tiles, x_tiles, m):
                ps = psb.tile([128, S], F32, name=f"{P}pp{dst[m].name}", tag="big")
                for sh in range(2):
                    for ci in range(3):
                        nc.tensor.matmul(
                            ps[:, sh * 512:(sh + 1) * 512],
                            w_tiles[ci][:, m * 128:(m + 1) * 128],
                            x_tiles[ci][:, sh * 512:(sh + 1) * 512],
                            start=(ci == 0),
                            stop=(ci == 2),
                        )
                nc.vector.tensor_copy(dst[m][:], ps[:])

            def attn(f, hp, extras=(), pre_pv=()):
                """One head-pair of one frame: scores -> exp -> PV -> normalize.
                `extras`: deferred closures (projection groups for later units)
                emitted one per tt so their psum use spreads across this unit."""
                extras = list(extras)
                oT = pout.tile([128, S], F32R, name=f"{P}outT{f}_{hp}", tag="oT", bufs=6)
                pv = [
                    [psv.tile([128, 512], F32, name=f"{P}pv{f}{hp}{par}{sh}", tag="pv") for sh in range(2)]
                    for par in range(2)
                ]
                for tt in range(8):
                    sts = [None, None]
                    for par in range(2):
                        hl = par * 64
                        st = psb.tile([128, S], F32, name=f"{P}st{f}{hp}{tt}{par}", tag="big")
                        sts[par] = st
                        for sh in range(2):
                            nc.tensor.matmul(
                                st[:, sh * 512:(sh + 1) * 512],
                                kTp[hp][hl:hl + 64, tt * 128:(tt + 1) * 128],
                                qTp[f][hp][hl:hl + 64, sh * 512:(sh + 1) * 512],
                                start=True,
                                stop=True,
                                tile_position=(hl, 0),
                            )
                    pts = [None, None]
                    for par in range(2):
                        pt = ppt.tile([128, S], F32R, name=f"{P}pt{f}{hp}{tt}{par}", tag="pt")
                        pts[par] = pt
                        nc.scalar.activation(
                            pt[:], sts[par][:], mybir.ActivationFunctionType.Exp, scale=SCALE
                        )
                    if tt < len(pre_pv):
                        pre_pv[tt]()
                    for par in range(2):
                        h = hp * 2 + par
                        for sh in range(2):
                            nc.tensor.matmul(
                                pv[par][sh][:],
                                v_sb[tt][:, h * 128:(h + 1) * 128],
                                pts[par][:, sh * 512:(sh + 1) * 512],
                                start=(tt == 0),
                                stop=(tt == 7),
                            )
                    if extras and tt % 2 == 0:
                        extras.pop(0)()
                for par in range(2):
                    for sh in range(2):
                        rc = prc.tile([64, 512], F32, name=f"{P}rc{f}{hp}{par}{sh}", tag="rc")
                        nc.vector.reciprocal(rc[:], pv[par][sh][64:128, :])
                        nc.vector.tensor_mul(
                            oT[par * 64:(par + 1) * 64, sh * 512:(sh + 1) * 512],
                            pv[par][sh][0:64, :],
                            rc[:],
                        )
                for ex in extras:
                    ex()
                return oT

            # ---- emission: short serial head (K0/Q00/V), then attention with
            # ---- remaining projection groups spread inside earlier units.
            import contextlib

            with tc.tile_pool(name=f"{P}pw", bufs=1) as pw:
                es = contextlib.ExitStack()
                px = es.enter_context(tc.tile_pool(name=f"{P}pxa", bufs=1))
                _, whot_v = one_dma_tile(pw, "whota", whot.ap(), 256)
                whot_k = [t[:, 0:128] for t in whot_v]
                whot_q = [t[:, 128:256] for t in whot_v]
                _, x0_sb = one_dma_tile(px, "x0a", xt0.ap(), S)
                xf_parts = [
                    pw.tile([128, 3 * S], F32R, name=f"{P}xfa{f}", tag=f"xfa{f}")
                    for f in range(FPC)
                ]
                xf_sb = [
                    [xf_parts[f][0:cn, ci * S:ci * S + S] for ci, (cs, cn) in enumerate(CI)]
                    for f in range(FPC)
                ]

                def load_xf_frame(f):
                    nc.sync.dma_start(
                        xf_parts[f][:].rearrange("p (c w) -> p c w", w=S),
                        xtf.ap()[f].rearrange("(c p) w -> p c w", p=128),
                    )

                load_xf_frame(0)
                _, wv_sb = one_dma_tile(pw, "wva", wvp.ap(), CP)
                _, wkq_v = one_dma_tile(pw, "wkqa", wkq.ap(), 2 * CP)
                wk_sb = [t[:, 0:CP] for t in wkq_v]
                wq_sb = [t[:, CP:2 * CP] for t in wkq_v]
                nc.sync.dma_start(
                    wo_all[:].rearrange("p (cp c) -> p cp c", c=C),
                    wo.ap().rearrange("(cp p) c -> p cp c", p=128),
                )
                bo_all = pconst.tile([128, 3], F32, name=f"{P}bo", tag="bo")
                nc.sync.dma_start(
                    bo_all[:],
                    bo.ap().rearrange("(c p) -> p c", p=128),
                )
                bo_sb = [bo_all[0:cn, m:m + 1] for m, (cs, cn) in enumerate(CO)]

                for sh in range(2):
                    ps = psb.tile([128, 512], F32, name=f"{P}ppk0{sh}", tag="big")
                    for ci in range(3):
                        nc.tensor.matmul(ps[:], whot_k[ci][:], x0_sb[ci][:, sh * 512:(sh + 1) * 512],
                                         start=(ci == 0), stop=(ci == 2))
                    nc.vector.tensor_copy(kTp[0][:, sh * 512:(sh + 1) * 512], ps[:])
                for sh in range(2):
                    ps = psb.tile([128, 512], F32, name=f"{P}ppq0{sh}", tag="big")
                    for ci in range(3):
                        nc.tensor.matmul(ps[:], whot_q[ci][:], xf_sb[0][ci][:, sh * 512:(sh + 1) * 512],
                                         start=(ci == 0), stop=(ci == 2))
                    nc.vector.tensor_copy(qTp[0][0][:, sh * 512:(sh + 1) * 512], ps[:])

                def vproj(tt):
                    ps = psb.tile([128, 512], F32, name=f"{P}ppv{tt}", tag="big")
                    for ci in range(3):
                        nc.tensor.matmul(
                            ps[:],
                            x0_sb[ci][:, tt * 128:(tt + 1) * 128],
                            wv_sb[ci][:],
                            start=(ci == 0),
                            stop=(ci == 2),
                        )
                    vview = v_sb[tt][:].rearrange("p (h c) -> p h c", c=128)
                    nc.vector.tensor_copy(
                        vview[:, :, 0:DP],
                        ps[:].rearrange("p (h c) -> p h c", c=DP),
                    )
                    nc.vector.tensor_copy(
                        vview[:, :, DP:128],
                        ones_sb[:].unsqueeze(1).broadcast_to([128, H, DP]),
                    )

                vproj(0)

                def pgroups(ms, dsts_w_x):
                    out = []
                    for m in ms:
                        for dst, w_t, x_t in dsts_w_x:
                            out.append(
                                (lambda dst=dst, w_t=w_t, x_t=x_t, m=m:
                                 proj_full(dst, w_t, x_t, m))
                            )
                    return out

                def oproj_groups(f, outTs, first, count):
                    """psum tiles allocated NOW (ordering!), matmuls deferred."""
                    groups = []
                    for g in range(first, first + count):
                        m, sh = divmod(g, 2)
                        cos, con = CO[m]
                        ps = psv.tile([con, 512], F32, name=f"{P}ppy{f}{m}{sh}", tag="pv")
                        y_sb = y_tiles[f][m]

                        def run(m=m, sh=sh, cos=cos, con=con, ps=ps, y_sb=y_sb, f=f):
                            for cp in range(4):
                                nc.tensor.matmul(
                                    ps[:],
                                    wo_sb[cp][:, cos:cos + con],
                                    outTs[cp][:, sh * 512:(sh + 1) * 512],
                                    start=(cp == 0),
                                    stop=(cp == 3),
                                )
                            nc.scalar.activation(
                                y_sb[:, sh * 512:(sh + 1) * 512], ps[:],
                                mybir.ActivationFunctionType.Identity, bias=bo_sb[m][:],
                            )
                            nc.sync.dma_start(
                                yt.ap()[f, cos:cos + con, sh * 512:(sh + 1) * 512],
                                y_sb[:, sh * 512:(sh + 1) * 512],
                            )

                        groups.append(run)
                    return groups

                y_tiles = [
                    [py.tile([CO[m][1], S], F32, name=f"{P}y{f}{m}", tag=f"y{m}", bufs=1) for m in range(3)]
                    for f in range(FPC)
                ]

                outT0 = [None] * 4
                outT1 = [None] * 4
                outT0[0] = attn(
                    0, 0,
                    pgroups([1], [(kTp, wk_sb, x0_sb), (qTp[0], wq_sb, xf_sb[0])]),
                    pre_pv=[(lambda tt=tt: vproj(tt + 1)) for tt in range(7)],
                )
                load_xf_frame(1)
                outT0[1] = attn(0, 1, pgroups([2], [(kTp, wk_sb, x0_sb), (qTp[0], wq_sb, xf_sb[0])]))
                outT0[2] = attn(0, 2, pgroups([3], [(kTp, wk_sb, x0_sb), (qTp[0], wq_sb, xf_sb[0])]))
                es.close()
                outT0[3] = attn(0, 3, pgroups([0, 1], [(qTp[1], wq_sb, xf_sb[1])]))
                outT1[0] = attn(1, 0, pgroups([2, 3], [(qTp[1], wq_sb, xf_sb[1])]))
                for run in oproj_groups(0, outT0, 0, 6):
                    run()
                outT1[1] = attn(1, 1)
                outT1[2] = attn(1, 2)
                outT1[3] = attn(1, 3)
                for run in oproj_groups(1, outT1, 0, 6):
                    run()

    nc.compile()
    return nc


def _get_nc(loop_n: int = 1):
    if loop_n not in _NC_CACHE:
        _NC_CACHE[loop_n] = _build(loop_n)
    return _NC_CACHE[loop_n]


def _pad_heads_cols(wT: np.ndarray) -> np.ndarray:
    """[C, C] (c_in, c_out) -> [C, CP] with each head's 40 cols at h*64."""
    out = np.zeros((C, CP), np.float32)
    out.reshape(C, H, DP)[:, :, :D] = wT.reshape(C, H, D)
    return out


def _prep_inputs(hidden_states, Wq, Wk, Wv, Wo, bo, video_length, k):
    hidden_states = np.asarray(hidden_states, dtype=np.float32)
    B = hidden_states.shape[0]
    assert hidden_states.shape == (B, S, C), hidden_states.shape
    assert B == NCORES * FPC, B
    kf = int(k)
    vl = int(video_length)
    b = B // vl
    assert b == 1, "kernel specialized for batch 1 (b*video_length == B)"

    xT = np.zeros((B, 384, S), np.float32)
    xT[:, :C, :] = hidden_states.transpose(0, 2, 1)
    wk_p = _pad_heads_cols(np.asarray(Wk, np.float32).T)
    wq_p = _pad_heads_cols(np.asarray(Wq, np.float32).T)
    wv_p = _pad_heads_cols(np.asarray(Wv, np.float32).T)
    wkq_p = np.zeros((384, 2 * CP), np.float32)
    wkq_p[:C] = np.concatenate([wk_p, wq_p], axis=1)
    wvp_p = np.zeros((384, CP), np.float32)
    wvp_p[:C] = wv_p
    whot_p = np.zeros((384, 256), np.float32)
    whot_p[:C] = np.concatenate([wk_p[:, 0:128], wq_p[:, 0:128]], axis=1)
    # WoT padded rows: row h*64+j = Wo[:, h*40+j]
    wo_p = np.zeros((CP, C), np.float32)
    wo_p.reshape(H, DP, C)[:, :D, :] = np.asarray(Wo, np.float32).T.reshape(H, D, C)
    bo_f = np.zeros(384, np.float32)
    bo_f[:C] = np.asarray(bo, np.float32)

    xt0 = np.ascontiguousarray(xT[kf])
    in_maps = []
    for c in range(NCORES):
        in_maps.append(
            {
                "xt0": xt0,
                "xtf": np.ascontiguousarray(xT[c * FPC:(c + 1) * FPC]),
                "wkq": wkq_p,
                "wvp": wvp_p,
                "whot": whot_p,
                "wo": wo_p,
                "bo": bo_f,
            }
        )
    return in_maps


def _run(inputs: dict, loop_n: int = 1):
    global LAST_RESULTS
    nc = _get_nc(loop_n)
    in_maps = _prep_inputs(**inputs)
    last_exc = None
    for _attempt in range(3):
        try:
            res = bass_utils.run_bass_kernel_spmd(nc, in_maps, core_ids=list(range(NCORES)))
            break
        except Exception as e:  # transient NRT/axon device hiccups
            last_exc = e
            import time as _time
            _time.sleep(2.0)
    else:
        raise last_exc
    LAST_RESULTS = res
    B = NCORES * FPC
    y = np.empty((B, S, C), np.float32)
    for c in range(NCORES):
        y[c * FPC:(c + 1) * FPC] = res.results[c]["yt"].transpose(0, 2, 1)
    return y


def kernel(hidden_states, Wq, Wk, Wv, Wo, bo, video_length, k):
    return _run(
        dict(
            hidden_states=hidden_states,
            Wq=Wq,
            Wk=Wk,
            Wv=Wv,
            Wo=Wo,
            bo=bo,
            video_length=video_length,
            k=k,
        )
    )



# revision 6
# speedup vs baseline: 1.0285x; 1.0285x over previous
"""Trainium2 Bass kernel for nn_BasicTransformerBlock (key-frame cross attention).

Reference computation (B=16 frames, S=1024, C=320, H=8 heads, D=40):
    q = x @ Wq.T ; k = x @ Wk.T ; v = x @ Wv.T
    k, v are taken from frame `kf` only and shared by every frame
    out = softmax(q k^T / sqrt(D)) v     (per frame, per head)
    y = out @ Wo.T + bo

Sharding: data-parallel over frames - 8 cores x 2 frames each. Every core
redundantly computes K/V from the key frame (cheap) so no collectives are
needed; outputs just concatenate.

Per-core design (v2, exp-wall balanced):
  - ScalarE (Act) does ONLY the 16.8M exps (2f x 8h x 1024^2 / core), in
    [128,1024] psum->sbuf passes; everything else is kept off that engine.
    That is the ~133us roofline of this kernel.
  - Scores run fp32r [t,s]-transposed exactly like the math: one 64-row
    head quadrant of kT against qT, N=512 halves (full PE rate).
  - PV runs TRANSPOSED: out[s,d] = sum_t pt[t,s] v[t,d] with pt (bf16, the
    exp output) as the stationary operand and v (bf16, 40 real dims + a
    ones column) as the moving operand. Matmul cost is (output free size) x
    (chunks), so N=41 instead of a 512-wide padded layout cuts PV cycles
    3x, and the ones column makes psum col 40 the softmax denominator.
  - Normalization is a [128,8] reciprocal plus one broadcast multiply per
    (frame,head) on DVE - per-partition scalars, no partition broadcasts.
  - o[s,d] is transposed back to oT[d,s] on the PE (is_transpose against a
    host-supplied identity), then the O-projection contracts the padded
    512 rows in fp32r; bias-add happens on DVE (tensor_scalar_add).
  - PSUM: 2x[128,1024] score tiles + 2x[128,328] pv+Z tiles + 2x[128,512]
    shared (projection groups / transposes / O-proj) = exactly 8 banks.
  - Projections (q/k/v) are fp32r and are drip-fed as deferred "extras"
    inside the attention unit loop so the PE never starves while Act
    grinds exps.
"""

import os
import sys

import numpy as np

try:
    import concourse  # noqa: F401
except ImportError:  # pragma: no cover
    for _p in ("/opt/trn_rl_repo", os.path.dirname(os.path.abspath(__file__))):
        if os.path.isdir(os.path.join(_p, "concourse")):
            sys.path.insert(0, _p)
            break

import concourse.mybir as mybir  # noqa: E402
import concourse.tile as tile  # noqa: E402
from concourse import bacc  # noqa: E402
from concourse import bass_utils  # noqa: E402

F32 = mybir.dt.float32
F32R = mybir.dt.float32r
BF16 = mybir.dt.bfloat16

S = 1024          # sequence length per frame
C = 320           # channels
H = 8             # heads
D = 40            # head dim
DP = 64           # padded head dim (q/k score layout + O-proj rows)
CP = H * DP       # 512
VW = D + 1        # v block width: 40 dims + ones column (denominator)
NCORES = 8
FPC = 2           # frames per core
SCALE = float(D) ** -0.5

CI = [(0, 128), (128, 128), (256, 64)]    # c_in chunks of 320
CO = [(0, 128), (128, 128), (256, 64)]    # c_out chunks of 320

_NC_CACHE: dict = {}
LAST_RESULTS = None


def _build(loop_n: int = 1):
    nc = bacc.Bacc("TRN2", target_bir_lowering=False, debug=False)

    CPAD = 384
    xt0 = nc.dram_tensor("xt0", [CPAD, S], F32R, kind="ExternalInput")
    xtf = nc.dram_tensor("xtf", [FPC, CPAD, S], F32R, kind="ExternalInput")
    wkq = nc.dram_tensor("wkq", [CPAD, 2 * CP], F32R, kind="ExternalInput")
    wvd = nc.dram_tensor("wvd", [CPAD, C], F32R, kind="ExternalInput")
    wo = nc.dram_tensor("wo", [CP, C], F32R, kind="ExternalInput")
    bo = nc.dram_tensor("bo", [CPAD], F32, kind="ExternalInput")
    ident = nc.dram_tensor("ident", [128, 128], F32R, kind="ExternalInput")
    yt = nc.dram_tensor("yt", [FPC, C, S], F32, kind="ExternalOutput")

    with tile.TileContext(nc) as tc:
        with (
            tc.tile_pool(name="pconst", bufs=1) as pconst,
            tc.tile_pool(name="pqk", bufs=1) as pqk,
            tc.tile_pool(name="pvs", bufs=1) as pvs,
            tc.tile_pool(name="ppt", bufs=2) as ppt,
            tc.tile_pool(name="po", bufs=2) as po,
            tc.tile_pool(name="pot", bufs=1) as pot,
            tc.tile_pool(name="prc", bufs=4) as prc,
            tc.tile_pool(name="py", bufs=2) as py,
            tc.tile_pool(name="psc", bufs=2, space="PSUM") as psc,
            tc.tile_pool(name="ppv", bufs=2, space="PSUM") as ppv,
            tc.tile_pool(name="psh", bufs=2, space="PSUM") as psh,
        ):
          for it in range(loop_n):
            P = f"{it}_"

            # ---- constants / inputs staged in SBUF --------------------
            ident_sb = pconst.tile([128, 128], F32R, name=f"{P}ident", tag="ident")
            nc.sync.dma_start(ident_sb[:], ident.ap())
            bo_all = pconst.tile([128, 3], F32, name=f"{P}bo", tag="bo")
            nc.sync.dma_start(bo_all[:], bo.ap().rearrange("(c p) -> p c", p=128))
            bo_sb = [bo_all[0:cn, m:m + 1] for m, (cs, cn) in enumerate(CO)]

            def one_dma_tile(pool, nm, dram_ap, width, dt=F32R):
                """[128, 3*width] tile <- [384, width] dram via one 3-level DMA."""
                t = pool.tile([128, 3 * width], dt, name=f"{P}{nm}", tag=nm)
                nc.sync.dma_start(
                    t[:].rearrange("p (c w) -> p c w", w=width),
                    dram_ap.rearrange("(c p) w -> p c w", p=128),
                )
                return t, [t[0:cn, ci * width:ci * width + width] for ci, (cs, cn) in enumerate(CI)]

            _, x0_sb = one_dma_tile(pconst, "x0a", xt0.ap(), S)
            _, wkq_v = one_dma_tile(pconst, "wkqa", wkq.ap(), 2 * CP)
            wk_sb = [t[:, 0:CP] for t in wkq_v]
            wq_sb = [t[:, CP:2 * CP] for t in wkq_v]
            _, wv_sb = one_dma_tile(pconst, "wva", wvd.ap(), C)
            wo_all = pconst.tile([128, 4 * C], F32R, name=f"{P}wo", tag="wo")
            nc.sync.dma_start(
                wo_all[:].rearrange("p (cp c) -> p cp c", c=C),
                wo.ap().rearrange("(cp p) c -> p cp c", p=128),
            )
            wo_sb = [wo_all[:, cp * C:(cp + 1) * C] for cp in range(4)]

            xf_parts = [
                pconst.tile([128, 3 * S], F32R, name=f"{P}xfa{f}", tag=f"xfa{f}")
                for f in range(FPC)
            ]
            xf_sb = [
                [xf_parts[f][0:cn, ci * S:ci * S + S] for ci, (cs, cn) in enumerate(CI)]
                for f in range(FPC)
            ]

            def load_xf_frame(f):
                nc.sync.dma_start(
                    xf_parts[f][:].rearrange("p (c w) -> p c w", w=S),
                    xtf.ap()[f].rearrange("(c p) w -> p c w", p=128),
                )

            load_xf_frame(0)

            # ---- persistent activations ------------------------------
            kTp = [pqk.tile([128, S], F32R, name=f"{P}kTp{m}", tag=f"kTp{m}") for m in range(4)]
            qTp = [
                [pqk.tile([128, S], F32R, name=f"{P}qTp{f}_{m}", tag="qT", bufs=8) for m in range(4)]
                for f in range(FPC)
            ]
            # v: [t, 8*(40+1)] bf16 per t-tile; col h*41+40 is the ones col
            v_sb = [pvs.tile([128, H * VW], BF16, name=f"{P}v{tt}", tag=f"v{tt}") for tt in range(8)]
            for tt in range(8):
                nc.gpsimd.memset(
                    v_sb[tt][:].rearrange("p (h w) -> p h w", w=VW)[:, :, D:VW], 1.0
                )
            # oT: [head-pair rows, s] per frame, padded 64/head
            oT = [
                [pot.tile([128, S], F32R, name=f"{P}oT{f}_{hp}", tag="oT", bufs=8) for hp in range(4)]
                for f in range(FPC)
            ]

            # ---- deferred projection groups --------------------------
            def proj_qk(dst_tiles, w_tiles, x_tiles, m):
                """One 128-row chunk of a [d-pad, s] projection (both halves)."""
                def run():
                    for sh in range(2):
                        ps = psh.tile([128, 512], F32, name=f"{P}pp{m}{sh}", tag="sh")
                        for ci in range(3):
                            nc.tensor.matmul(
                                ps[:],
                                w_tiles[ci][:, m * 128:(m + 1) * 128],
                                x_tiles[ci][:, sh * 512:(sh + 1) * 512],
                                start=(ci == 0),
                                stop=(ci == 2),
                            )
                        nc.vector.tensor_copy(dst_tiles[m][:, sh * 512:(sh + 1) * 512], ps[:])
                return run

            def proj_v(tt):
                """v[t, c-dense] for one t-tile; bf16 copy into 41-strided blocks."""
                def run():
                    ps = psh.tile([128, 512], F32, name=f"{P}ppv{tt}", tag="sh")
                    for ci in range(3):
                        nc.tensor.matmul(
                            ps[0:128, 0:C],
                            x0_sb[ci][:, tt * 128:(tt + 1) * 128],
                            wv_sb[ci][:],
                            start=(ci == 0),
                            stop=(ci == 2),
                        )
                    nc.vector.tensor_copy(
                        v_sb[tt][:].rearrange("p (h w) -> p h w", w=VW)[:, :, 0:D],
                        ps[0:128, 0:C].rearrange("p (h d) -> p h d", d=D),
                    )
                return run

            extras = []
            extras.append(proj_qk(kTp, wk_sb, x0_sb, 1))
            extras.append(proj_qk(qTp[0], wq_sb, xf_sb[0], 1))
            for tt in range(1, 8):
                extras.append(proj_v(tt))
            for m in (2, 3):
                extras.append(proj_qk(kTp, wk_sb, x0_sb, m))
                extras.append(proj_qk(qTp[0], wq_sb, xf_sb[0], m))

            def pop_extra():
                if extras:
                    extras.pop(0)()

            # minimal serial head: k m0, q f0 m0, v tt0
            proj_qk(kTp, wk_sb, x0_sb, 0)()
            proj_qk(qTp[0], wq_sb, xf_sb[0], 0)()
            proj_v(0)()

            # ---- attention units -------------------------------------
            def unit(f, h):
                hp, hl = h // 2, (h % 2) * 64
                pvz = ppv.tile([128, 8 * VW], F32, name=f"{P}pvz{f}{h}", tag="pvz")
                pt_u = ppt.tile([128, 8 * S], BF16, name=f"{P}pt{f}{h}", tag="pt")
                for tt in range(8):
                    st = psc.tile([128, S], F32, name=f"{P}st{f}{h}{tt}", tag="sc")
                    for sh in range(2):
                        nc.tensor.matmul(
                            st[:, sh * 512:(sh + 1) * 512],
                            kTp[hp][hl:hl + 64, tt * 128:(tt + 1) * 128],
                            qTp[f][hp][hl:hl + 64, sh * 512:(sh + 1) * 512],
                            start=True,
                            stop=True,
                            tile_position=(hl, 0),
                        )
                    nc.scalar.activation(
                        pt_u[:, tt * S:(tt + 1) * S], st[:],
                        mybir.ActivationFunctionType.Exp, scale=SCALE,
                    )
                    for sb in range(8):
                        nc.tensor.matmul(
                            pvz[:, sb * VW:(sb + 1) * VW],
                            pt_u[:, tt * S + sb * 128:tt * S + (sb + 1) * 128],
                            v_sb[tt][:, h * VW:(h + 1) * VW],
                            start=(tt == 0),
                            stop=(tt == 7),
                        )
                    pop_extra()
                # normalize: per-s reciprocal of Z (col 40 of each block)
                pvz_v = pvz[:].rearrange("p (b w) -> p b w", w=VW)
                rcz = prc.tile([128, 8], F32, name=f"{P}rcz{f}{h}", tag="rcz")
                nc.vector.reciprocal(rcz[:], pvz_v[:, :, D:VW].rearrange("p b w -> p (b w)"))
                o_u = po.tile([128, 8 * DP], F32R, name=f"{P}o{f}{h}", tag="o")
                o_v = o_u[:].rearrange("p (b d) -> p b d", d=DP)
                # pad cols must be finite: the matching wo rows are zero
                nc.gpsimd.memset(o_v[:, :, D:DP], 0.0)
                nc.vector.tensor_mul(
                    o_v[:, :, 0:D],
                    pvz_v[:, :, 0:D],
                    rcz[:].unsqueeze(2).broadcast_to([128, 8, D]),
                )
                # transpose o[s, d] -> oT[d, s] (8 s-blocks, 2 psum tiles)
                for half in range(2):
                    tp = psh.tile([128, 512], F32R, name=f"{P}tp{f}{h}{half}", tag="sh")
                    for b in range(4):
                        sb = half * 4 + b
                        nc.tensor.transpose(
                            tp[0:64, b * 128:(b + 1) * 128],
                            o_u[:, sb * DP:sb * DP + DP],
                            ident_sb[:],
                        )
                    nc.vector.tensor_copy(
                        oT[f][hp][hl:hl + 64, half * 512:(half + 1) * 512],
                        tp[0:64, :],
                    )

            def oproj(f):
                for m in range(3):
                    cos, con = CO[m]
                    for sh in range(2):
                        ps = psh.tile([con, 512], F32, name=f"{P}py{f}{m}{sh}", tag="sh")
                        for cp in range(4):
                            nc.tensor.matmul(
                                ps[:],
                                wo_sb[cp][:, cos:cos + con],
                                oT[f][cp][:, sh * 512:(sh + 1) * 512],
                                start=(cp == 0),
                                stop=(cp == 3),
                            )
                        y_sb = py.tile([con, 512], F32, name=f"{P}y{f}{m}{sh}", tag="y")
                        nc.vector.tensor_scalar_add(y_sb[:], ps[:], bo_sb[m][:])
                        nc.sync.dma_start(
                            yt.ap()[f, cos:cos + con, sh * 512:(sh + 1) * 512],
                            y_sb[:],
                        )

            for h in range(H):
                unit(0, h)
                if h == 0:
                    load_xf_frame(1)
                    for m in range(4):
                        extras.append(proj_qk(qTp[1], wq_sb, xf_sb[1], m))
            oproj(0)
            for h in range(H):
                unit(1, h)
            oproj(1)
            while extras:
                pop_extra()

    nc.compile()
    return nc


def _get_nc(loop_n: int = 1):
    if loop_n not in _NC_CACHE:
        _NC_CACHE[loop_n] = _build(loop_n)
    return _NC_CACHE[loop_n]


def _pad_heads_cols(wT: np.ndarray) -> np.ndarray:
    """[C, C] (c_in, c_out) -> [C, CP] with each head's 40 cols at h*64."""
    out = np.zeros((C, CP), np.float32)
    out.reshape(C, H, DP)[:, :, :D] = wT.reshape(C, H, D)
    return out


def _prep_inputs(hidden_states, Wq, Wk, Wv, Wo, bo, video_length, k):
    hidden_states = np.asarray(hidden_states, dtype=np.float32)
    B = hidden_states.shape[0]
    assert hidden_states.shape == (B, S, C), hidden_states.shape
    assert B == NCORES * FPC, B
    kf = int(k)
    vl = int(video_length)
    b = B // vl
    assert b == 1, "kernel specialized for batch 1 (b*video_length == B)"

    xT = np.zeros((B, 384, S), np.float32)
    xT[:, :C, :] = hidden_states.transpose(0, 2, 1)
    wk_p = _pad_heads_cols(np.asarray(Wk, np.float32).T)
    wq_p = _pad_heads_cols(np.asarray(Wq, np.float32).T)
    wkq_p = np.zeros((384, 2 * CP), np.float32)
    wkq_p[:C] = np.concatenate([wk_p, wq_p], axis=1)
    wvd_p = np.zeros((384, C), np.float32)
    wvd_p[:C] = np.asarray(Wv, np.float32).T
    wo_p = np.zeros((CP, C), np.float32)
    wo_p.reshape(H, DP, C)[:, :D, :] = np.asarray(Wo, np.float32).T.reshape(H, D, C)
    bo_f = np.zeros(384, np.float32)
    bo_f[:C] = np.asarray(bo, np.float32)
    ident = np.eye(128, dtype=np.float32)

    xt0 = np.ascontiguousarray(xT[kf])
    in_maps = []
    for c in range(NCORES):
        in_maps.append(
            {
                "xt0": xt0,
                "xtf": np.ascontiguousarray(xT[c * FPC:(c + 1) * FPC]),
                "wkq": wkq_p,
                "wvd": wvd_p,
                "wo": wo_p,
                "bo": bo_f,
                "ident": ident,
            }
        )
    return in_maps


def _run(inputs: dict, loop_n: int = 1):
    global LAST_RESULTS
    nc = _get_nc(loop_n)
    in_maps = _prep_inputs(**inputs)
    last_exc = None
    for _attempt in range(3):
        try:
            res = bass_utils.run_bass_kernel_spmd(nc, in_maps, core_ids=list(range(NCORES)))
            break
        except Exception as e:  # transient NRT/axon device hiccups
            last_exc = e
            import time as _time
            _time.sleep(2.0)
    else:
        raise last_exc
    LAST_RESULTS = res
    B = NCORES * FPC
    y = np.empty((B, S, C), np.float32)
    for c in range(NCORES):
        y[c * FPC:(c + 1) * FPC] = res.results[c]["yt"].transpose(0, 2, 1)
    return y


def kernel(hidden_states, Wq, Wk, Wv, Wo, bo, video_length, k):
    return _run(
        dict(
            hidden_states=hidden_states,
            Wq=Wq,
            Wk=Wk,
            Wv=Wv,
            Wo=Wo,
            bo=bo,
            video_length=video_length,
            k=k,
        )
    )


# revision 7
# speedup vs baseline: 1.1338x; 1.1024x over previous
"""Trainium2 Bass kernel for nn_BasicTransformerBlock (key-frame cross attention).

Reference computation (B=16 frames, S=1024, C=320, H=8 heads, D=40):
    q = x @ Wq.T ; k = x @ Wk.T ; v = x @ Wv.T
    k, v are taken from frame `kf` only and shared by every frame
    out = softmax(q k^T / sqrt(D)) v     (per frame, per head)
    y = out @ Wo.T + bo

Sharding: data-parallel over frames - 8 cores x 2 frames each. Every core
redundantly computes K/V from the key frame (cheap) so no collectives are
needed; outputs just concatenate.

Per-core design (v2, exp-wall balanced):
  - ScalarE (Act) does ONLY the 16.8M exps (2f x 8h x 1024^2 / core), in
    [128,1024] psum->sbuf passes; everything else is kept off that engine.
    That is the ~133us roofline of this kernel.
  - Scores run fp32r [t,s]-transposed exactly like the math: one 64-row
    head quadrant of kT against qT, N=512 halves (full PE rate).
  - PV runs TRANSPOSED: out[s,d] = sum_t pt[t,s] v[t,d] with pt (bf16, the
    exp output) as the stationary operand and v (bf16, 40 real dims + a
    ones column) as the moving operand. Matmul cost is (output free size) x
    (chunks), so N=41 instead of a 512-wide padded layout cuts PV cycles
    3x, and the ones column makes psum col 40 the softmax denominator.
  - Normalization is a [128,8] reciprocal plus one broadcast multiply per
    (frame,head) on DVE - per-partition scalars, no partition broadcasts.
  - o[s,d] is transposed back to oT[d,s] on the PE (is_transpose against a
    host-supplied identity), then the O-projection contracts the padded
    512 rows in fp32r; bias-add happens on DVE (tensor_scalar_add).
  - PSUM: 2x[128,1024] score tiles + 2x[128,328] pv+Z tiles + 2x[128,512]
    shared (projection groups / transposes / O-proj) = exactly 8 banks.
  - Projections (q/k/v) are fp32r and are drip-fed as deferred "extras"
    inside the attention unit loop so the PE never starves while Act
    grinds exps.
"""

import os
import sys

import numpy as np

try:
    import concourse  # noqa: F401
except ImportError:  # pragma: no cover
    for _p in ("/opt/trn_rl_repo", os.path.dirname(os.path.abspath(__file__))):
        if os.path.isdir(os.path.join(_p, "concourse")):
            sys.path.insert(0, _p)
            break

import concourse.mybir as mybir  # noqa: E402
import concourse.tile as tile  # noqa: E402
from concourse import bacc  # noqa: E402
from concourse import bass_utils  # noqa: E402

F32 = mybir.dt.float32
F32R = mybir.dt.float32r
BF16 = mybir.dt.bfloat16

S = 1024          # sequence length per frame
C = 320           # channels
H = 8             # heads
D = 40            # head dim
DP = 64           # padded head dim (q/k score layout + O-proj rows)
CP = H * DP       # 512
VW = D + 1        # v block width: 40 dims + ones column (denominator)
NCORES = 8
FPC = 2           # frames per core
SCALE = float(D) ** -0.5

CI = [(0, 128), (128, 128), (256, 64)]    # c_in chunks of 320
CO = [(0, 128), (128, 128), (256, 64)]    # c_out chunks of 320

_NC_CACHE: dict = {}
LAST_RESULTS = None


def _build(loop_n: int = 1):
    nc = bacc.Bacc("TRN2", target_bir_lowering=False, debug=False)

    CPAD = 384
    xt0 = nc.dram_tensor("xt0", [CPAD, S], F32R, kind="ExternalInput")
    xtf = nc.dram_tensor("xtf", [FPC, CPAD, S], F32R, kind="ExternalInput")
    wkq = nc.dram_tensor("wkq", [CPAD, 2 * CP], F32R, kind="ExternalInput")
    wvd = nc.dram_tensor("wvd", [CPAD, C], F32R, kind="ExternalInput")
    wo = nc.dram_tensor("wo", [CP, C], F32R, kind="ExternalInput")
    bo = nc.dram_tensor("bo", [CPAD], F32, kind="ExternalInput")
    ident = nc.dram_tensor("ident", [128, 128], F32R, kind="ExternalInput")
    yt = nc.dram_tensor("yt", [FPC, C, S], F32, kind="ExternalOutput")

    with tile.TileContext(nc) as tc:
        with (
            tc.tile_pool(name="pconst", bufs=1) as pconst,
            tc.tile_pool(name="pqk", bufs=1) as pqk,
            tc.tile_pool(name="pvs", bufs=1) as pvs,
            tc.tile_pool(name="ppt", bufs=2) as ppt,
            tc.tile_pool(name="po", bufs=2) as po,
            tc.tile_pool(name="pot", bufs=1) as pot,
            tc.tile_pool(name="prc", bufs=4) as prc,
            tc.tile_pool(name="py", bufs=2) as py,
            tc.tile_pool(name="psc", bufs=2, space="PSUM") as psc,
            tc.tile_pool(name="ppv", bufs=2, space="PSUM") as ppv,
            tc.tile_pool(name="psh", bufs=2, space="PSUM") as psh,
        ):
          for it in range(loop_n):
            P = f"{it}_"

            # ---- constants / inputs staged in SBUF --------------------
            ident_sb = pconst.tile([128, 128], F32R, name=f"{P}ident", tag="ident")
            nc.sync.dma_start(ident_sb[:], ident.ap())
            bo_all = pconst.tile([128, 3], F32, name=f"{P}bo", tag="bo")
            nc.sync.dma_start(bo_all[:], bo.ap().rearrange("(c p) -> p c", p=128))
            bo_sb = [bo_all[0:cn, m:m + 1] for m, (cs, cn) in enumerate(CO)]

            def one_dma_tile(pool, nm, dram_ap, width, dt=F32R):
                """[128, 3*width] tile <- [384, width] dram via one 3-level DMA."""
                t = pool.tile([128, 3 * width], dt, name=f"{P}{nm}", tag=nm)
                nc.sync.dma_start(
                    t[:].rearrange("p (c w) -> p c w", w=width),
                    dram_ap.rearrange("(c p) w -> p c w", p=128),
                )
                return t, [t[0:cn, ci * width:ci * width + width] for ci, (cs, cn) in enumerate(CI)]

            _, x0_sb = one_dma_tile(pconst, "x0a", xt0.ap(), S)
            _, wkq_v = one_dma_tile(pconst, "wkqa", wkq.ap(), 2 * CP)
            wk_sb = [t[:, 0:CP] for t in wkq_v]
            wq_sb = [t[:, CP:2 * CP] for t in wkq_v]
            _, wv_sb = one_dma_tile(pconst, "wva", wvd.ap(), C)
            wo_all = pconst.tile([128, 4 * C], F32R, name=f"{P}wo", tag="wo")
            nc.sync.dma_start(
                wo_all[:].rearrange("p (cp c) -> p cp c", c=C),
                wo.ap().rearrange("(cp p) c -> p cp c", p=128),
            )
            wo_sb = [wo_all[:, cp * C:(cp + 1) * C] for cp in range(4)]

            xf_parts = [
                pconst.tile([128, 3 * S], F32R, name=f"{P}xfa{f}", tag=f"xfa{f}")
                for f in range(FPC)
            ]
            xf_sb = [
                [xf_parts[f][0:cn, ci * S:ci * S + S] for ci, (cs, cn) in enumerate(CI)]
                for f in range(FPC)
            ]

            def load_xf_frame(f):
                nc.sync.dma_start(
                    xf_parts[f][:].rearrange("p (c w) -> p c w", w=S),
                    xtf.ap()[f].rearrange("(c p) w -> p c w", p=128),
                )

            load_xf_frame(0)

            # ---- persistent activations ------------------------------
            kTp = [pqk.tile([128, S], F32R, name=f"{P}kTp{m}", tag=f"kTp{m}") for m in range(4)]
            qTp = [
                [pqk.tile([128, S], F32R, name=f"{P}qTp{f}_{m}", tag="qT", bufs=8) for m in range(4)]
                for f in range(FPC)
            ]
            # v: [t, 8*(40+1)] bf16 per t-tile; col h*41+40 is the ones col
            v_sb = [pvs.tile([128, H * VW], BF16, name=f"{P}v{tt}", tag=f"v{tt}") for tt in range(8)]
            for tt in range(8):
                nc.gpsimd.memset(
                    v_sb[tt][:].rearrange("p (h w) -> p h w", w=VW)[:, :, D:VW], 1.0
                )
            # oT: [head-pair rows, s] per frame, padded 64/head
            oT = [
                [pot.tile([128, S], F32R, name=f"{P}oT{f}_{hp}", tag="oT", bufs=8) for hp in range(4)]
                for f in range(FPC)
            ]

            # ---- deferred projection groups --------------------------
            def proj_qk(dst_tiles, w_tiles, x_tiles, m):
                """One 128-row chunk of a [d-pad, s] projection (both halves)."""
                def run():
                    for sh in range(2):
                        ps = psh.tile([128, 512], F32, name=f"{P}pp{m}{sh}", tag="sh")
                        for ci in range(3):
                            nc.tensor.matmul(
                                ps[:],
                                w_tiles[ci][:, m * 128:(m + 1) * 128],
                                x_tiles[ci][:, sh * 512:(sh + 1) * 512],
                                start=(ci == 0),
                                stop=(ci == 2),
                            )
                        nc.vector.tensor_copy(dst_tiles[m][:, sh * 512:(sh + 1) * 512], ps[:])
                return run

            def proj_v(tt):
                """v[t, c-dense] for one t-tile; bf16 copy into 41-strided blocks."""
                def run():
                    ps = psh.tile([128, 512], F32, name=f"{P}ppv{tt}", tag="sh")
                    for ci in range(3):
                        nc.tensor.matmul(
                            ps[0:128, 0:C],
                            x0_sb[ci][:, tt * 128:(tt + 1) * 128],
                            wv_sb[ci][:],
                            start=(ci == 0),
                            stop=(ci == 2),
                        )
                    nc.vector.tensor_copy(
                        v_sb[tt][:].rearrange("p (h w) -> p h w", w=VW)[:, :, 0:D],
                        ps[0:128, 0:C].rearrange("p (h d) -> p h d", d=D),
                    )
                return run

            extras = []
            extras.append(proj_qk(kTp, wk_sb, x0_sb, 1))
            extras.append(proj_qk(qTp[0], wq_sb, xf_sb[0], 1))
            for tt in range(1, 8):
                extras.append(proj_v(tt))
            for m in (2, 3):
                extras.append(proj_qk(kTp, wk_sb, x0_sb, m))
                extras.append(proj_qk(qTp[0], wq_sb, xf_sb[0], m))

            def pop_extra():
                if extras:
                    extras.pop(0)()

            # minimal serial head: k m0, q f0 m0, v tt0
            proj_qk(kTp, wk_sb, x0_sb, 0)()
            proj_qk(qTp[0], wq_sb, xf_sb[0], 0)()
            proj_v(0)()

            # ---- attention: software-pipelined (unit, tt) stream -----
            # PE is strictly in-order, so scores are emitted 2 steps ahead
            # of their exp, and all other PE work (PV epilogues, O-proj,
            # projection extras) is drip-fed between steps so the PE never
            # parks behind an Act dependency.
            steps = [(f, h, tt) for f in range(FPC) for h in range(H) for tt in range(8)]
            st_tiles: dict = {}
            pvz_tiles: dict = {}
            pt_tiles: dict = {}

            def emit_scores(i):
                f, h, tt = steps[i]
                hp, hl = h // 2, (h % 2) * 64
                st = psc.tile([128, S], F32, name=f"{P}st{f}{h}{tt}", tag="sc")
                st_tiles[i] = st
                for sh in range(2):
                    nc.tensor.matmul(
                        st[:, sh * 512:(sh + 1) * 512],
                        kTp[hp][hl:hl + 64, tt * 128:(tt + 1) * 128],
                        qTp[f][hp][hl:hl + 64, sh * 512:(sh + 1) * 512],
                        start=True,
                        stop=True,
                        tile_position=(hl, 0),
                    )

            def transposes_half(f, h, o_u, half):
                hp, hl = h // 2, (h % 2) * 64
                tp = psh.tile([128, 512], F32R, name=f"{P}tp{f}{h}{half}", tag="sh")
                for b in range(4):
                    sb = half * 4 + b
                    nc.tensor.transpose(
                        tp[0:64, b * 128:(b + 1) * 128],
                        o_u[:, sb * DP:sb * DP + DP],
                        ident_sb[:],
                    )
                nc.vector.tensor_copy(
                    oT[f][hp][hl:hl + 64, half * 512:(half + 1) * 512],
                    tp[0:64, :],
                )

            def oproj_group(f, m, sh):
                cos, con = CO[m]
                ps = psh.tile([con, 512], F32, name=f"{P}py{f}{m}{sh}", tag="sh")
                for cp in range(4):
                    nc.tensor.matmul(
                        ps[:],
                        wo_sb[cp][:, cos:cos + con],
                        oT[f][cp][:, sh * 512:(sh + 1) * 512],
                        start=(cp == 0),
                        stop=(cp == 3),
                    )
                y_sb = py.tile([con, 512], F32, name=f"{P}y{f}{m}{sh}", tag="y")
                nc.vector.tensor_scalar_add(y_sb[:], ps[:], bo_sb[m][:])
                nc.sync.dma_start(
                    yt.ap()[f, cos:cos + con, sh * 512:(sh + 1) * 512],
                    y_sb[:],
                )

            emit_scores(0)
            emit_scores(1)
            for i, (f, h, tt) in enumerate(steps):
                if tt == 0:
                    pvz_tiles[(f, h)] = ppv.tile(
                        [128, 8 * VW], F32, name=f"{P}pvz{f}{h}", tag="pvz"
                    )
                    pt_tiles[(f, h)] = ppt.tile(
                        [128, 8 * S], BF16, name=f"{P}pt{f}{h}", tag="pt"
                    )
                st = st_tiles.pop(i)
                pt_u = pt_tiles[(f, h)]
                pvz = pvz_tiles[(f, h)]
                nc.scalar.activation(
                    pt_u[:, tt * S:(tt + 1) * S], st[:],
                    mybir.ActivationFunctionType.Exp, scale=SCALE,
                )
                if i + 2 < len(steps):
                    emit_scores(i + 2)
                for sb in range(8):
                    nc.tensor.matmul(
                        pvz[:, sb * VW:(sb + 1) * VW],
                        pt_u[:, tt * S + sb * 128:tt * S + (sb + 1) * 128],
                        v_sb[tt][:, h * VW:(h + 1) * VW],
                        start=(tt == 0),
                        stop=(tt == 7),
                    )
                if tt == 7:
                    # normalize now (DVE only - does not block the PE queue)
                    pvz_v = pvz[:].rearrange("p (b w) -> p b w", w=VW)
                    rcz = prc.tile([128, 8], F32, name=f"{P}rcz{f}{h}", tag="rcz")
                    nc.vector.reciprocal(
                        rcz[:], pvz_v[:, :, D:VW].rearrange("p b w -> p (b w)")
                    )
                    o_u = po.tile([128, 8 * DP], F32R, name=f"{P}o{f}{h}", tag="o")
                    o_v = o_u[:].rearrange("p (b d) -> p b d", d=DP)
                    nc.gpsimd.memset(o_v[:, :, D:DP], 0.0)
                    nc.vector.tensor_mul(
                        o_v[:, :, 0:D],
                        pvz_v[:, :, 0:D],
                        rcz[:].unsqueeze(2).broadcast_to([128, 8, D]),
                    )
                    del pvz_tiles[(f, h)], pt_tiles[(f, h)]
                    # PE-side epilogue is deferred a couple of steps
                    extras.append(lambda f=f, h=h, o_u=o_u: transposes_half(f, h, o_u, 0))
                    extras.append(lambda f=f, h=h, o_u=o_u: transposes_half(f, h, o_u, 1))
                    if h == H - 1:
                        for m in range(3):
                            for sh in range(2):
                                extras.append(lambda f=f, m=m, sh=sh: oproj_group(f, m, sh))
                    if f == 0 and h == 0:
                        load_xf_frame(1)
                        for m in range(4):
                            extras.append(proj_qk(qTp[1], wq_sb, xf_sb[1], m))
                else:
                    pop_extra()
            while extras:
                pop_extra()

    nc.compile()
    return nc


def _get_nc(loop_n: int = 1):
    if loop_n not in _NC_CACHE:
        _NC_CACHE[loop_n] = _build(loop_n)
    return _NC_CACHE[loop_n]


def _pad_heads_cols(wT: np.ndarray) -> np.ndarray:
    """[C, C] (c_in, c_out) -> [C, CP] with each head's 40 cols at h*64."""
    out = np.zeros((C, CP), np.float32)
    out.reshape(C, H, DP)[:, :, :D] = wT.reshape(C, H, D)
    return out


def _prep_inputs(hidden_states, Wq, Wk, Wv, Wo, bo, video_length, k):
    hidden_states = np.asarray(hidden_states, dtype=np.float32)
    B = hidden_states.shape[0]
    assert hidden_states.shape == (B, S, C), hidden_states.shape
    assert B == NCORES * FPC, B
    kf = int(k)
    vl = int(video_length)
    b = B // vl
    assert b == 1, "kernel specialized for batch 1 (b*video_length == B)"

    xT = np.zeros((B, 384, S), np.float32)
    xT[:, :C, :] = hidden_states.transpose(0, 2, 1)
    wk_p = _pad_heads_cols(np.asarray(Wk, np.float32).T)
    wq_p = _pad_heads_cols(np.asarray(Wq, np.float32).T)
    wkq_p = np.zeros((384, 2 * CP), np.float32)
    wkq_p[:C] = np.concatenate([wk_p, wq_p], axis=1)
    wvd_p = np.zeros((384, C), np.float32)
    wvd_p[:C] = np.asarray(Wv, np.float32).T
    wo_p = np.zeros((CP, C), np.float32)
    wo_p.reshape(H, DP, C)[:, :D, :] = np.asarray(Wo, np.float32).T.reshape(H, D, C)
    bo_f = np.zeros(384, np.float32)
    bo_f[:C] = np.asarray(bo, np.float32)
    ident = np.eye(128, dtype=np.float32)

    xt0 = np.ascontiguousarray(xT[kf])
    in_maps = []
    for c in range(NCORES):
        in_maps.append(
            {
                "xt0": xt0,
                "xtf": np.ascontiguousarray(xT[c * FPC:(c + 1) * FPC]),
                "wkq": wkq_p,
                "wvd": wvd_p,
                "wo": wo_p,
                "bo": bo_f,
                "ident": ident,
            }
        )
    return in_maps


def _run(inputs: dict, loop_n: int = 1):
    global LAST_RESULTS
    nc = _get_nc(loop_n)
    in_maps = _prep_inputs(**inputs)
    last_exc = None
    for _attempt in range(3):
        try:
            res = bass_utils.run_bass_kernel_spmd(nc, in_maps, core_ids=list(range(NCORES)))
            break
        except Exception as e:  # transient NRT/axon device hiccups
            last_exc = e
            import time as _time
            _time.sleep(2.0)
    else:
        raise last_exc
    LAST_RESULTS = res
    B = NCORES * FPC
    y = np.empty((B, S, C), np.float32)
    for c in range(NCORES):
        y[c * FPC:(c + 1) * FPC] = res.results[c]["yt"].transpose(0, 2, 1)
    return y


def kernel(hidden_states, Wq, Wk, Wv, Wo, bo, video_length, k):
    return _run(
        dict(
            hidden_states=hidden_states,
            Wq=Wq,
            Wk=Wk,
            Wv=Wv,
            Wo=Wo,
            bo=bo,
            video_length=video_length,
            k=k,
        )
    )


# revision 13
# speedup vs baseline: 1.2031x; 1.0612x over previous
"""Trainium2 Bass kernel for nn_BasicTransformerBlock (key-frame cross attention).

Reference computation (B=16 frames, S=1024, C=320, H=8 heads, D=40):
    q = x @ Wq.T ; k = x @ Wk.T ; v = x @ Wv.T
    k, v are taken from frame `kf` only and shared by every frame
    out = softmax(q k^T / sqrt(D)) v     (per frame, per head)
    y = out @ Wo.T + bo

Sharding: data-parallel over frames - 8 cores x 2 frames each. Every core
redundantly computes K/V from the key frame (cheap) so no collectives are
needed; outputs just concatenate.

Per-core design (v2, exp-wall balanced):
  - ScalarE (Act) does ONLY the 16.8M exps (2f x 8h x 1024^2 / core), in
    [128,1024] psum->sbuf passes; everything else is kept off that engine.
    That is the ~133us roofline of this kernel.
  - Scores run fp32r [t,s]-transposed exactly like the math: one 64-row
    head quadrant of kT against qT, N=512 halves (full PE rate).
  - PV runs TRANSPOSED: out[s,d] = sum_t pt[t,s] v[t,d] with pt (bf16, the
    exp output) as the stationary operand and v (bf16, 40 real dims + a
    ones column) as the moving operand. Matmul cost is (output free size) x
    (chunks), so N=41 instead of a 512-wide padded layout cuts PV cycles
    3x, and the ones column makes psum col 40 the softmax denominator.
  - Normalization is a [128,8] reciprocal plus one broadcast multiply per
    (frame,head) on DVE - per-partition scalars, no partition broadcasts.
  - o[s,d] is transposed back to oT[d,s] on the PE (is_transpose against a
    host-supplied identity), then the O-projection contracts the padded
    512 rows in fp32r; bias-add happens on DVE (tensor_scalar_add).
  - PSUM: 2x[128,1024] score tiles + 2x[128,328] pv+Z tiles + 2x[128,512]
    shared (projection groups / transposes / O-proj) = exactly 8 banks.
  - Projections (q/k/v) are fp32r and are drip-fed as deferred "extras"
    inside the attention unit loop so the PE never starves while Act
    grinds exps.
"""

import os
import sys

import numpy as np

try:
    import concourse  # noqa: F401
except ImportError:  # pragma: no cover
    for _p in ("/opt/trn_rl_repo", os.path.dirname(os.path.abspath(__file__))):
        if os.path.isdir(os.path.join(_p, "concourse")):
            sys.path.insert(0, _p)
            break

import concourse.mybir as mybir  # noqa: E402
import concourse.tile as tile  # noqa: E402
from concourse import bacc  # noqa: E402
from concourse import bass_utils  # noqa: E402

F32 = mybir.dt.float32
F32R = mybir.dt.float32r
BF16 = mybir.dt.bfloat16

S = 1024          # sequence length per frame
C = 320           # channels
H = 8             # heads
D = 40            # head dim
DP = 64           # padded head dim (q/k score layout + O-proj rows)
CP = H * DP       # 512
VW = D + 1        # v block width: 40 dims + ones column (denominator)
NCORES = 8
FPC = 2           # frames per core
SCALE = float(D) ** -0.5

CI = [(0, 128), (128, 128), (256, 64)]    # c_in chunks of 320
CO = [(0, 128), (128, 128), (256, 64)]    # c_out chunks of 320

_NC_CACHE: dict = {}
LAST_RESULTS = None


def _build(loop_n: int = 1):
    nc = bacc.Bacc("TRN2", target_bir_lowering=False, debug=False)

    CPAD = 384
    xt0 = nc.dram_tensor("xt0", [CPAD, S], F32R, kind="ExternalInput")
    xtf = nc.dram_tensor("xtf", [FPC, CPAD, S], F32R, kind="ExternalInput")
    wkq = nc.dram_tensor("wkq", [CPAD, 2 * CP], F32R, kind="ExternalInput")
    wvd = nc.dram_tensor("wvd", [CPAD, C], F32R, kind="ExternalInput")
    wo = nc.dram_tensor("wo", [CP, C], F32R, kind="ExternalInput")
    bo = nc.dram_tensor("bo", [CPAD], F32, kind="ExternalInput")
    ident = nc.dram_tensor("ident", [128, 128], F32R, kind="ExternalInput")
    yt = nc.dram_tensor("yt", [FPC, C, S], F32, kind="ExternalOutput")

    with tile.TileContext(nc) as tc:
        with (
            tc.tile_pool(name="pconst", bufs=1) as pconst,
            tc.tile_pool(name="pqk", bufs=1) as pqk,
            tc.tile_pool(name="pvs", bufs=1) as pvs,
            tc.tile_pool(name="ppt", bufs=2) as ppt,
            tc.tile_pool(name="po", bufs=2) as po,
            tc.tile_pool(name="pot", bufs=1) as pot,
            tc.tile_pool(name="prc", bufs=4) as prc,
            tc.tile_pool(name="py", bufs=2) as py,
            tc.tile_pool(name="psc", bufs=2, space="PSUM") as psc,
            tc.tile_pool(name="ppv", bufs=2, space="PSUM") as ppv,
            tc.tile_pool(name="psh", bufs=2, space="PSUM") as psh,
        ):
          for it in range(loop_n):
            P = f"{it}_"

            # ---- constants / inputs staged in SBUF --------------------
            ident_sb = pconst.tile([128, 128], F32R, name=f"{P}ident", tag="ident")
            bo_all = pconst.tile([128, 3], F32, name=f"{P}bo", tag="bo")
            bo_sb = [bo_all[0:cn, m:m + 1] for m, (cs, cn) in enumerate(CO)]

            def dma_cols(tile_t, dram_ap, width, c0, c1):
                """Columns [c0:c1] of a [384, width] dram tensor into the
                matching slice of a [128, 3*width] folded tile."""
                nc.sync.dma_start(
                    tile_t[:].rearrange("p (c w) -> p c w", w=width)[:, :, c0:c1],
                    dram_ap.rearrange("(c p) w -> p c w", p=128)[:, :, c0:c1],
                )

            def mk_tile3(pool, nm, width, dt=F32R):
                t = pool.tile([128, 3 * width], dt, name=f"{P}{nm}", tag=nm)
                return t, [t[0:cn, ci * width:ci * width + width] for ci, (cs, cn) in enumerate(CI)]

            x0_t, x0_sb = mk_tile3(pconst, "x0a", S)
            wkq_t, wkq_v = mk_tile3(pconst, "wkqa", 2 * CP)
            wk_sb = [t[:, 0:CP] for t in wkq_v]
            wq_sb = [t[:, CP:2 * CP] for t in wkq_v]
            wv_t, wv_sb = mk_tile3(pconst, "wva", C)
            wo_all = pconst.tile([128, 4 * C], F32R, name=f"{P}wo", tag="wo")
            wo_sb = [wo_all[:, cp * C:(cp + 1) * C] for cp in range(4)]

            xf_parts = [
                pconst.tile([128, 3 * S], F32R, name=f"{P}xfa{f}", tag=f"xfa{f}")
                for f in range(FPC)
            ]
            xf_sb = [
                [xf_parts[f][0:cn, ci * S:ci * S + S] for ci, (cs, cn) in enumerate(CI)]
                for f in range(FPC)
            ]

            def load_xf_frame(f):
                nc.sync.dma_start(
                    xf_parts[f][:].rearrange("p (c w) -> p c w", w=S),
                    xtf.ap()[f].rearrange("(c p) w -> p c w", p=128),
                )

            # DMA issue order = the startup critical path: the q side
            # (xf0 + wq-m0) and k side (x0 t-chunks + wk-m0) race so the
            # first exp fires ~7us in; everything else lands behind them.
            load_xf_frame(0)
            dma_cols(wkq_t, wkq.ap(), 2 * CP, CP, CP + 128)      # wq m0
            dma_cols(wkq_t, wkq.ap(), 2 * CP, 0, 128)            # wk m0
            for tc in range(4):                                   # x0 t-chunks
                dma_cols(x0_t, xt0.ap(), S, tc * 256, (tc + 1) * 256)
            dma_cols(wv_t, wvd.ap(), C, 0, C)                     # wv
            dma_cols(wkq_t, wkq.ap(), 2 * CP, 128, CP)            # wk rest
            dma_cols(wkq_t, wkq.ap(), 2 * CP, CP + 128, 2 * CP)   # wq rest
            nc.sync.dma_start(
                wo_all[:].rearrange("p (cp c) -> p cp c", c=C),
                wo.ap().rearrange("(cp p) c -> p cp c", p=128),
            )
            nc.sync.dma_start(ident_sb[:], ident.ap())
            nc.sync.dma_start(bo_all[:], bo.ap().rearrange("(c p) -> p c", p=128))

            # ---- persistent activations ------------------------------
            kTp = [pqk.tile([128, S], F32R, name=f"{P}kTp{m}", tag=f"kTp{m}") for m in range(4)]
            qTp = [
                [pqk.tile([128, S], F32R, name=f"{P}qTp{f}_{m}", tag="qT", bufs=8) for m in range(4)]
                for f in range(FPC)
            ]
            # v: [t, 8*(40+1)] bf16 per t-tile; col h*41+40 is the ones col
            v_sb = [pvs.tile([128, H * VW], BF16, name=f"{P}v{tt}", tag=f"v{tt}") for tt in range(8)]
            for tt in range(8):
                nc.gpsimd.memset(
                    v_sb[tt][:].rearrange("p (h w) -> p h w", w=VW)[:, :, D:VW], 1.0
                )
            # oT: [head-pair rows, s] per frame, padded 64/head
            oT = [
                [pot.tile([128, S], F32R, name=f"{P}oT{f}_{hp}", tag="oT", bufs=8) for hp in range(4)]
                for f in range(FPC)
            ]

            # ---- deferred projection groups --------------------------
            def proj_qk_half(dst_tiles, w_tiles, x_tiles, m, sh):
                """One 512-col half of a 128-row [d-pad, s] projection chunk."""
                def run():
                    ps = psh.tile([128, 512], F32, name=f"{P}pp{m}{sh}", tag="sh")
                    for ci in range(3):
                        nc.tensor.matmul(
                            ps[:],
                            w_tiles[ci][:, m * 128:(m + 1) * 128],
                            x_tiles[ci][:, sh * 512:(sh + 1) * 512],
                            start=(ci == 0),
                            stop=(ci == 2),
                        )
                    nc.vector.tensor_copy(dst_tiles[m][:, sh * 512:(sh + 1) * 512], ps[:])
                return run

            def proj_qk(dst_tiles, w_tiles, x_tiles, m):
                h0 = proj_qk_half(dst_tiles, w_tiles, x_tiles, m, 0)
                h1 = proj_qk_half(dst_tiles, w_tiles, x_tiles, m, 1)
                def run():
                    h0()
                    h1()
                return run

            def proj_v(tt):
                """v[t, c-dense] for one t-tile; bf16 copy into 41-strided blocks."""
                def run():
                    ps = psh.tile([128, 512], F32, name=f"{P}ppv{tt}", tag="sh")
                    for ci in range(3):
                        nc.tensor.matmul(
                            ps[0:128, 0:C],
                            x0_sb[ci][:, tt * 128:(tt + 1) * 128],
                            wv_sb[ci][:],
                            start=(ci == 0),
                            stop=(ci == 2),
                        )
                    nc.vector.tensor_copy(
                        v_sb[tt][:].rearrange("p (h w) -> p h w", w=VW)[:, :, 0:D],
                        ps[0:128, 0:C].rearrange("p (h d) -> p h d", d=D),
                    )
                return run

            extras = []
            for sh in range(2):
                extras.append(proj_qk_half(kTp, wk_sb, x0_sb, 1, sh))
                extras.append(proj_qk_half(qTp[0], wq_sb, xf_sb[0], 1, sh))
            for tt in range(1, 8):
                extras.append(proj_v(tt))
            for m in (2, 3):
                for sh in range(2):
                    extras.append(proj_qk_half(kTp, wk_sb, x0_sb, m, sh))
                    extras.append(proj_qk_half(qTp[0], wq_sb, xf_sb[0], m, sh))

            def pop_extra():
                if extras:
                    extras.pop(0)()

            # ---- serial head, staged to chase the DMA chunks ----------
            # q f0 m0: both halves (needs all of xf0 + wq-m0)
            proj_qk(qTp[0], wq_sb, xf_sb[0], 0)()
            # k m0 in four 256-col t-chunks so scores(tt) can start as the
            # matching x0 t-chunk DMA lands instead of waiting for all of x0
            for tc in range(4):
                ps = psh.tile([128, 512], F32, name=f"{P}ppk0_{tc}", tag="sh")
                for ci in range(3):
                    nc.tensor.matmul(
                        ps[:, 0:256],
                        wk_sb[ci][:, 0:128],
                        x0_sb[ci][:, tc * 256:(tc + 1) * 256],
                        start=(ci == 0),
                        stop=(ci == 2),
                    )
                nc.vector.tensor_copy(kTp[0][:, tc * 256:(tc + 1) * 256], ps[:, 0:256])
            proj_v(0)()

            # ---- attention: software-pipelined (unit, tt) stream -----
            # PE is strictly in-order, so scores are emitted 2 steps ahead
            # of their exp, and all other PE work (PV epilogues, O-proj,
            # projection extras) is drip-fed between steps so the PE never
            # parks behind an Act dependency.
            steps = [(f, h, tt) for f in range(FPC) for h in range(H) for tt in range(8)]
            st_tiles: dict = {}
            pvz_tiles: dict = {}
            pt_tiles: dict = {}

            def emit_scores(i):
                f, h, tt = steps[i]
                hp, hl = h // 2, (h % 2) * 64
                st = psc.tile([128, S], F32, name=f"{P}st{f}{h}{tt}", tag="sc")
                st_tiles[i] = st
                for sh in range(2):
                    nc.tensor.matmul(
                        st[:, sh * 512:(sh + 1) * 512],
                        kTp[hp][hl:hl + 64, tt * 128:(tt + 1) * 128],
                        qTp[f][hp][hl:hl + 64, sh * 512:(sh + 1) * 512],
                        start=True,
                        stop=True,
                        tile_position=(hl, 0),
                    )

            def transposes_half(f, h, o_u, half):
                hp, hl = h // 2, (h % 2) * 64
                tp = psh.tile([128, 512], F32R, name=f"{P}tp{f}{h}{half}", tag="sh")
                for b in range(4):
                    sb = half * 4 + b
                    nc.tensor.transpose(
                        tp[0:64, b * 128:(b + 1) * 128],
                        o_u[:, sb * DP:sb * DP + DP],
                        ident_sb[:],
                    )
                nc.vector.tensor_copy(
                    oT[f][hp][hl:hl + 64, half * 512:(half + 1) * 512],
                    tp[0:64, :],
                )

            def oproj_group(f, m, sh):
                cos, con = CO[m]
                ps = psh.tile([con, 512], F32, name=f"{P}py{f}{m}{sh}", tag="sh")
                for cp in range(4):
                    nc.tensor.matmul(
                        ps[:],
                        wo_sb[cp][:, cos:cos + con],
                        oT[f][cp][:, sh * 512:(sh + 1) * 512],
                        start=(cp == 0),
                        stop=(cp == 3),
                    )
                y_sb = py.tile([con, 512], F32, name=f"{P}y{f}{m}{sh}", tag="y")
                if f == FPC - 1:
                    # tail: Act is idle after the last exp - use it
                    nc.scalar.activation(
                        y_sb[:], ps[:],
                        mybir.ActivationFunctionType.Identity, bias=bo_sb[m][:],
                    )
                else:
                    nc.vector.tensor_scalar_add(y_sb[:], ps[:], bo_sb[m][:])
                nc.sync.dma_start(
                    yt.ap()[f, cos:cos + con, sh * 512:(sh + 1) * 512],
                    y_sb[:],
                )

            emit_scores(0)
            emit_scores(1)
            for i, (f, h, tt) in enumerate(steps):
                if tt == 0:
                    pvz_tiles[(f, h)] = ppv.tile(
                        [128, 8 * VW], F32, name=f"{P}pvz{f}{h}", tag="pvz"
                    )
                    pt_tiles[(f, h)] = ppt.tile(
                        [128, 8 * S], BF16, name=f"{P}pt{f}{h}", tag="pt"
                    )
                st = st_tiles.pop(i)
                pt_u = pt_tiles[(f, h)]
                pvz = pvz_tiles[(f, h)]
                nc.scalar.activation(
                    pt_u[:, tt * S:(tt + 1) * S], st[:],
                    mybir.ActivationFunctionType.Exp, scale=SCALE,
                )
                if i + 2 < len(steps):
                    emit_scores(i + 2)
                for sb in range(8):
                    nc.tensor.matmul(
                        pvz[:, sb * VW:(sb + 1) * VW],
                        pt_u[:, tt * S + sb * 128:tt * S + (sb + 1) * 128],
                        v_sb[tt][:, h * VW:(h + 1) * VW],
                        start=(tt == 0),
                        stop=(tt == 7),
                    )
                if tt == 7:
                    # normalize now (DVE only - does not block the PE queue)
                    pvz_v = pvz[:].rearrange("p (b w) -> p b w", w=VW)
                    rcz = prc.tile([128, 8], F32, name=f"{P}rcz{f}{h}", tag="rcz")
                    nc.vector.reciprocal(
                        rcz[:], pvz_v[:, :, D:VW].rearrange("p b w -> p (b w)")
                    )
                    o_u = po.tile([128, 8 * DP], F32R, name=f"{P}o{f}{h}", tag="o")
                    o_v = o_u[:].rearrange("p (b d) -> p b d", d=DP)
                    nc.gpsimd.memset(o_v[:, :, D:DP], 0.0)
                    nc.vector.tensor_mul(
                        o_v[:, :, 0:D],
                        pvz_v[:, :, 0:D],
                        rcz[:].unsqueeze(2).broadcast_to([128, 8, D]),
                    )
                    del pvz_tiles[(f, h)], pt_tiles[(f, h)]
                    # PE-side epilogue is deferred a couple of steps
                    extras.append(lambda f=f, h=h, o_u=o_u: transposes_half(f, h, o_u, 0))
                    extras.append(lambda f=f, h=h, o_u=o_u: transposes_half(f, h, o_u, 1))
                    if h == H - 1:
                        for m in range(3):
                            for sh in range(2):
                                extras.append(lambda f=f, m=m, sh=sh: oproj_group(f, m, sh))
                    if f == 0 and h == 0:
                        load_xf_frame(1)
                        for m in range(4):
                            extras.append(proj_qk(qTp[1], wq_sb, xf_sb[1], m))
                else:
                    pop_extra()
            while extras:
                pop_extra()

    nc.compile()
    return nc


def _get_nc(loop_n: int = 1):
    if loop_n not in _NC_CACHE:
        _NC_CACHE[loop_n] = _build(loop_n)
    return _NC_CACHE[loop_n]


def _pad_heads_cols(wT: np.ndarray) -> np.ndarray:
    """[C, C] (c_in, c_out) -> [C, CP] with each head's 40 cols at h*64."""
    out = np.zeros((C, CP), np.float32)
    out.reshape(C, H, DP)[:, :, :D] = wT.reshape(C, H, D)
    return out


def _prep_inputs(hidden_states, Wq, Wk, Wv, Wo, bo, video_length, k):
    hidden_states = np.asarray(hidden_states, dtype=np.float32)
    B = hidden_states.shape[0]
    assert hidden_states.shape == (B, S, C), hidden_states.shape
    assert B == NCORES * FPC, B
    kf = int(k)
    vl = int(video_length)
    b = B // vl
    assert b == 1, "kernel specialized for batch 1 (b*video_length == B)"

    xT = np.zeros((B, 384, S), np.float32)
    xT[:, :C, :] = hidden_states.transpose(0, 2, 1)
    wk_p = _pad_heads_cols(np.asarray(Wk, np.float32).T)
    wq_p = _pad_heads_cols(np.asarray(Wq, np.float32).T)
    wkq_p = np.zeros((384, 2 * CP), np.float32)
    wkq_p[:C] = np.concatenate([wk_p, wq_p], axis=1)
    wvd_p = np.zeros((384, C), np.float32)
    wvd_p[:C] = np.asarray(Wv, np.float32).T
    wo_p = np.zeros((CP, C), np.float32)
    wo_p.reshape(H, DP, C)[:, :D, :] = np.asarray(Wo, np.float32).T.reshape(H, D, C)
    bo_f = np.zeros(384, np.float32)
    bo_f[:C] = np.asarray(bo, np.float32)
    ident = np.eye(128, dtype=np.float32)

    xt0 = np.ascontiguousarray(xT[kf])
    in_maps = []
    for c in range(NCORES):
        in_maps.append(
            {
                "xt0": xt0,
                "xtf": np.ascontiguousarray(xT[c * FPC:(c + 1) * FPC]),
                "wkq": wkq_p,
                "wvd": wvd_p,
                "wo": wo_p,
                "bo": bo_f,
                "ident": ident,
            }
        )
    return in_maps


def _run(inputs: dict, loop_n: int = 1):
    global LAST_RESULTS
    nc = _get_nc(loop_n)
    in_maps = _prep_inputs(**inputs)
    last_exc = None
    for _attempt in range(3):
        try:
            res = bass_utils.run_bass_kernel_spmd(nc, in_maps, core_ids=list(range(NCORES)))
            break
        except Exception as e:  # transient NRT/axon device hiccups
            last_exc = e
            import time as _time
            _time.sleep(2.0)
    else:
        raise last_exc
    LAST_RESULTS = res
    B = NCORES * FPC
    y = np.empty((B, S, C), np.float32)
    for c in range(NCORES):
        y[c * FPC:(c + 1) * FPC] = res.results[c]["yt"].transpose(0, 2, 1)
    return y


def kernel(hidden_states, Wq, Wk, Wv, Wo, bo, video_length, k):
    return _run(
        dict(
            hidden_states=hidden_states,
            Wq=Wq,
            Wk=Wk,
            Wv=Wv,
            Wo=Wo,
            bo=bo,
            video_length=video_length,
            k=k,
        )
    )
